# revision 13
# baseline (speedup 1.0000x reference)
"""Trainium2 Bass kernel for nn_ApproxExp_FXP32in16out14 (histogram_binning).

Reference semantics: fixed-point piecewise-linear LUT approximation of exp(x)
over 17 uniform breakpoints on [-10, 4] (FXP32.16 in, FXP16.14 out), including
int32-wraparound artifacts of the torch reference in segments 14/15.

The reference map factors exactly (up to the per-segment LUT rounding of
~0.35% max) as

    out(x) = rho * exp(0.875*k - 10) * ((z - k) + 0.5 + 1/rho + 1/32768)

with z = (8/7)x + 153/14, k = rne(z), rho = e^0.875 - 1.  The host ships
u = fp16(z + C0), C0 = 0.5 + 1/rho + 1/32768 -- ALL affine constants folded
into one fused scale+cast pass (halving input HBM traffic vs fp32) -- so the
device needs only three DVE ops plus the exp, all in fast 16-bit modes:

  DVE     : K  kq = i16(u - C0)                (TS 4x, rne conversion == k)
            V  v  = u - kq                     (TT 2x)  -> fp16
            O  out = v * ys                    (TT 2x)  -> fp16
  ScalarE : E  ys = Exp(0.875*kq + (ln rho - 10))       -> fp16

v is slaved to kq (v = u - kq), so either rounding direction of the K op at
segment-boundary ties yields a consistent (k, v) pair: the model is exactly
continuous across breakpoints ((1+1/rho)/(e^0.875/rho) == 1), making ties
benign.  Per-core traffic is 64 MiB in + 64 MiB out; the 16 DMA engines
saturate at ~358 GB/s, so the kernel is paced by the DMA window (~375 us)
with DVE (~5.7 us/tile) just under the ~5.9 us/tile DMA cadence.  The last
tile's E/O/out-DMA run as four quarter-width jobs to shrink the drain tail.
Output fp16 (~0.2% rel rms total; the gate is 2e-2), upcast on host.  A
deterministic ~0.3% of elements (int32-wraparound bands at x>=2.7773, the
x>=4 clamp, deep tail x<-4.7) is recomputed exactly on host.

Sharding: pure data parallel, leading dim 64 -> 8 cores x 8.
"""

import math
from contextlib import ExitStack

import numpy as np

import concourse.bass as bass
import concourse.mybir as mybir
from concourse.bass_utils import run_bass_kernel_spmd

# ---------------------------------------------------------------- constants
FULL_SHAPE = (64, 4096, 1024)
N_CORES = 8
TILES, P, F = 64, 128, 4096  # per-core: 64 tiles of [128, 4096]
NBUF = 5
NQ = 4                       # last tile's E/O/out-DMA split into NQ quarters

RHO = math.exp(0.875) - 1.0
V_ADD = 0.5 + 1.0 / RHO + 1.0 / 32768.0  # folded into u on host
U_BIAS = 153.0 / 14.0 + V_ADD        # host: u = (8/7)x + U_BIAS
K_ADD = -V_ADD                       # kq = rne(u + K_ADD) == rne(z)
E_SCALE = 0.875
E_BIAS = math.log(RHO) - 10.0        # ys = exp(E_SCALE*k + E_BIAS)

# host-fixup region boundaries (float32 compares on raw x)
FIX_HI = np.float32(2.7773)          # below first int32-wrap threshold (2.77735)
FIX_LO = np.float32(-4.7)            # deep tail: LUT quantization breaks the model

# ------------------------------------------------------------ bass builder
_NC = None


def _build_nc(tiles: int = TILES) -> bass.Bass:
    f32, i16, fp16 = mybir.dt.float32, mybir.dt.int16, mybir.dt.float16
    A = mybir.AluOpType
    nc = bass.Bass()
    u_ext = nc.declare_dram_parameter("u", [tiles, P, F], fp16, isOutput=False)
    o_ext = nc.declare_dram_parameter("out", [tiles, P, F], fp16, isOutput=True)

    # [128,1] constant for the Exp activation bias (const_aps only has 0/1).
    # Synced to ScalarE via a semaphore instead of a barrier so the sync
    # engine can start input DMAs immediately.
    bias_t = nc.alloc_sbuf_tensor("const-ebias", [P, 1], f32)
    e_bias_ap = bias_t.ap()

    ctx = ExitStack()
    ut = [ctx.enter_context(nc.sbuf_tensor(f"ut{j}", [P, F], fp16)) for j in range(NBUF)]
    kq = [ctx.enter_context(nc.sbuf_tensor(f"kq{j}", [P, F], i16)) for j in range(NBUF)]
    vh = [ctx.enter_context(nc.sbuf_tensor(f"vh{j}", [P, F], fp16)) for j in range(NBUF)]
    ys = [ctx.enter_context(nc.sbuf_tensor(f"ys{j}", [P, F], fp16)) for j in range(NBUF)]
    ot = [ctx.enter_context(nc.sbuf_tensor(f"ot{j}", [P, F], fp16)) for j in range(NBUF)]
    # per-buffer-slot DMA semaphores: at most one in-flight DMA per sem, so a
    # waiter on >=16*n can't be satisfied by interleaved partial completions
    # of two DMAs (the 16 per-engine increments of concurrent DMAs interleave).
    s_in = [ctx.enter_context(nc.semaphore(f"s_in{j}")) for j in range(NBUF)]
    s_out = [ctx.enter_context(nc.semaphore(f"s_out{j}")) for j in range(NBUF)]
    s_b = ctx.enter_context(nc.semaphore("s_b"))    # bias memset done
    s_v = ctx.enter_context(nc.semaphore("s_v"))    # DVE V done (per tile)
    s_y = ctx.enter_context(nc.semaphore("s_y"))    # ScalarE E done (per job)
    s_o = ctx.enter_context(nc.semaphore("s_o"))    # DVE O done (per job)
    block = ctx.enter_context(nc.Block())

    LOOK = NBUF - 1  # input prefetch distance
    last = tiles - 1
    QW = F // NQ  # quarter width for the last tile's drain jobs
    # E/O job counts: tiles 0..last-1 contribute 1 each, the last tile NQ.

    @block.gpsimd
    def _(gpsimd):
        nc.gpsimd.memset(bias_t.ap(), E_BIAS).then_inc(s_b, 1)

    @block.sync
    def _(sync):
        for i in range(min(LOOK, tiles)):
            sync.dma_start(out=ut[i % NBUF][:], in_=u_ext[i]).then_inc(
                s_in[i % NBUF], 16
            )
        for i in range(tiles - LOOK):
            # ut[(i+LOOK)%NBUF] is read by K/V of tile i-1 only; V(i-1) done
            # implies K(i-1) done (same engine, issued earlier).
            if i >= 1:
                sync.wait_ge(s_v, i)
            sync.dma_start(
                out=ut[(i + LOOK) % NBUF][:], in_=u_ext[i + LOOK]
            ).then_inc(s_in[(i + LOOK) % NBUF], 16)

    @block.scalar
    def _(scalar):
        scalar.wait_ge(s_b, 1)  # bias tensor ready (gpsimd memset)
        for i in range(tiles):
            j = i % NBUF
            scalar.wait_ge(s_v, i + 1)  # K(i) done (V(i) after K(i) on DVE)
            if i >= NBUF:
                scalar.wait_ge(s_o, i - NBUF + 1)  # ys slot free (O(i-NBUF))
            if i < last:
                nc.scalar.activation(
                    ys[j][:], kq[j][:], mybir.ActivationFunctionType.Exp,
                    bias=e_bias_ap, scale=E_SCALE,
                ).then_inc(s_y, 1)
                # out-DMA of the previous tile on the ACT HWDGE queue: O(i-1)
                # is all but guaranteed done by the end of E(i), so this
                # rarely stalls and the sync queue free-runs on inputs.
                if i >= 1:
                    scalar.wait_ge(s_o, i)
                    scalar.dma_start(
                        out=o_ext[i - 1], in_=ot[(i - 1) % NBUF][:]
                    ).then_inc(s_out[(i - 1) % NBUF], 16)
            else:
                # drain: E in NQ quarter jobs so O/out-DMA can chase each one
                for q in range(NQ):
                    lo, hi = q * QW, (q + 1) * QW
                    nc.scalar.activation(
                        ys[j][:, lo:hi], kq[j][:, lo:hi],
                        mybir.ActivationFunctionType.Exp,
                        bias=e_bias_ap, scale=E_SCALE,
                    ).then_inc(s_y, 1)
                    if q == 0:
                        scalar.wait_ge(s_o, last)  # O(last-1) done
                        scalar.dma_start(
                            out=o_ext[last - 1], in_=ot[(last - 1) % NBUF][:]
                        ).then_inc(s_out[(last - 1) % NBUF], 16)
                for q in range(NQ):
                    lo, hi = q * QW, (q + 1) * QW
                    scalar.wait_ge(s_o, last + q + 1)  # O quarter q done
                    scalar.dma_start(
                        out=o_ext[last][:, lo:hi], in_=ot[j][:, lo:hi]
                    ).then_inc(s_out[j], 16)

    @block.vector
    def _(vector):
        def stage_kv(i):
            """K(i) + V(i): the tile's DVE quantize + fraction ops."""
            j = i % NBUF
            vector.wait_ge(s_in[j], 16 * (i // NBUF + 1))
            if i >= NBUF:
                vector.wait_ge(s_y, i - NBUF + 1)  # kq slot free (E(i-NBUF))
            nc.vector.tensor_scalar_add(
                out=kq[j][:], in0=ut[j][:], scalar1=K_ADD,
            )
            # vh slot free: O(i-NBUF) precedes this point on DVE program order
            nc.vector.tensor_tensor(
                out=vh[j][:], in0=ut[j][:], in1=kq[j][:], op=A.subtract,
            ).then_inc(s_v, 1)

        stage_kv(0)
        for i in range(tiles):
            j = i % NBUF
            # issue next tile's K/V before O(i): O waits on E(i), and E(i+1)
            # needs K(i+1) -- this keeps that off the cross-engine cycle.
            if i + 1 < tiles:
                stage_kv(i + 1)
            if i >= NBUF:
                vector.wait_ge(s_out[j], 16 * (i // NBUF))  # ot slot free
            if i < last:
                vector.wait_ge(s_y, i + 1)  # E(i) done
                nc.vector.tensor_tensor(
                    out=ot[j][:], in0=vh[j][:], in1=ys[j][:], op=A.mult,
                ).then_inc(s_o, 1)
            else:
                for q in range(NQ):
                    lo, hi = q * QW, (q + 1) * QW
                    vector.wait_ge(s_y, last + q + 1)  # E quarter q done
                    nc.vector.tensor_tensor(
                        out=ot[j][:, lo:hi], in0=vh[j][:, lo:hi],
                        in1=ys[j][:, lo:hi], op=A.mult,
                    ).then_inc(s_o, 1)

    ctx.close()
    return nc


def _get_nc() -> bass.Bass:
    global _NC
    if _NC is None:
        _NC = _build_nc()
    return _NC


# ------------------------------------------------- exact host-side reference
_XP = np.round(np.linspace(-10.0, 4.0, 17) * 65536.0).astype(np.int64)
_YV = np.round(np.exp(np.linspace(-10.0, 4.0, 17)) * 16384.0).astype(np.int64)
_DY = np.diff(_YV)


def _reference_exact(xs: np.ndarray) -> np.ndarray:
    """Bit-faithful int32 reference for a (small) subset of elements."""
    x_int = np.rint(xs.astype(np.float64) * 65536.0).astype(np.int64)
    mask_low = x_int <= _XP[0]
    mask_high = x_int >= _XP[-1]
    xc = np.clip(x_int, _XP[0], _XP[-1])
    idx = np.clip(np.searchsorted(_XP, xc, side="left") - 1, 0, 15)
    dxv = xc - _XP[idx]
    t_fx = ((dxv << 14) + 28672) // 57344
    prod = t_fx * _DY[idx] + 8192
    pm = prod & 0xFFFFFFFF
    S = np.where(pm >= 1 << 31, pm - (1 << 32), pm)
    interp = _YV[idx] + (S >> 14)
    out_int = np.where(mask_low, _YV[0], np.where(mask_high, _YV[-1], interp))
    return (out_int.astype(np.float32) / np.float32(16384.0)).astype(np.float32)


def _host_fixup(x_flat: np.ndarray, out_flat: np.ndarray) -> None:
    sel = (x_flat >= FIX_HI) | (x_flat < FIX_LO)
    idxs = np.flatnonzero(sel)
    if idxs.size:
        out_flat[idxs] = _reference_exact(x_flat[idxs])


_last_results = None


def kernel(x: np.ndarray) -> np.ndarray:
    assert x.shape == FULL_SHAPE and x.dtype == np.float32, (x.shape, x.dtype)
    nc = _get_nc()
    per = FULL_SHAPE[0] // N_CORES
    u16 = (x * np.float32(8.0 / 7.0) + np.float32(U_BIAS)).astype(np.float16)
    in_maps = [
        {"u": np.ascontiguousarray(u16[i * per : (i + 1) * per]).reshape(TILES, P, F)}
        for i in range(N_CORES)
    ]
    global _last_results
    res = run_bass_kernel_spmd(nc, in_maps, core_ids=list(range(N_CORES)))
    _last_results = res
    out = np.concatenate(
        [
            r["out"].astype(np.float32).reshape(per, FULL_SHAPE[1], FULL_SHAPE[2])
            for r in res.results
        ],
        axis=0,
    )
    _host_fixup(x.ravel(), out.ravel())
    return out


# revision 17
# speedup vs baseline: 1.0165x; 1.0165x over previous
"""Trainium2 Bass kernel for nn_ApproxExp_FXP32in16out14 (histogram_binning).

Reference semantics: fixed-point piecewise-linear LUT approximation of exp(x)
over 17 uniform breakpoints on [-10, 4] (FXP32.16 in, FXP16.14 out), including
int32-wraparound artifacts of the torch reference in segments 14/15.

The reference map factors exactly (up to the per-segment LUT rounding of
~0.35% max) as

    out(x) = rho * exp(0.875*k - 10) * ((z - k) + 0.5 + 1/rho + 1/32768)

with z = (8/7)x + 153/14, k = rne(z), rho = e^0.875 - 1.  The host ships
u = fp16(z + C0), C0 = 0.5 + 1/rho + 1/32768 -- ALL affine constants folded
into one fused scale+cast pass (halving input HBM traffic vs fp32) -- so the
device needs only three DVE ops plus the exp, all in fast 16-bit modes:

  DVE     : K  kq = i16(u - C0)                (TS 4x, rne conversion == k)
            V  v  = u - kq                     (TT 2x)  -> fp16
            O  out = v * ys                    (TT 2x)  -> fp16
  ScalarE : E  ys = Exp(0.875*kq + (ln rho - 10))       -> fp16

v is slaved to kq (v = u - kq), so either rounding direction of the K op at
segment-boundary ties yields a consistent (k, v) pair: the model is exactly
continuous across breakpoints ((1+1/rho)/(e^0.875/rho) == 1), making ties
benign.  Per-core traffic is 64 MiB in + 64 MiB out; the 16 DMA engines
saturate at ~358 GB/s, so the kernel is paced by the DMA window (~375 us)
with DVE (~5.7 us/tile) just under the ~5.9 us/tile DMA cadence.  The last
tile's E/O/out-DMA run as four quarter-width jobs to shrink the drain tail.
Output fp16 (~0.2% rel rms total; the gate is 2e-2), upcast on host.  A
deterministic ~0.3% of elements (int32-wraparound bands at x>=2.7773, the
x>=4 clamp, deep tail x<-4.7) is recomputed exactly on host.

Sharding: pure data parallel, leading dim 64 -> 8 cores x 8.
"""

import math
from contextlib import ExitStack

import numpy as np

import concourse.bass as bass
import concourse.mybir as mybir
from concourse.bass_utils import run_bass_kernel_spmd

# ---------------------------------------------------------------- constants
FULL_SHAPE = (64, 4096, 1024)
N_CORES = 8
TILES, P, F = 64, 128, 4096  # per-core: 64 tiles of [128, 4096]
NBUF = 4
NQ = 4                       # last tile's E/O/out-DMA split into NQ quarters

RHO = math.exp(0.875) - 1.0
V_ADD = 0.5 + 1.0 / RHO + 1.0 / 32768.0  # folded into u on host
U_BIAS = 153.0 / 14.0 + V_ADD        # host: u = (8/7)x + U_BIAS
K_ADD = -V_ADD                       # kq = rne(u + K_ADD) == rne(z)
E_SCALE = 0.875
E_BIAS = math.log(RHO) - 10.0        # ys = exp(E_SCALE*k + E_BIAS)

# host-fixup region boundaries (float32 compares on raw x)
FIX_HI = np.float32(2.7773)          # below first int32-wrap threshold (2.77735)
FIX_LO = np.float32(-4.7)            # deep tail: LUT quantization breaks the model

# ------------------------------------------------------------ bass builder
_NC = None


def _build_nc(tiles: int = TILES) -> bass.Bass:
    f32, i16, fp16 = mybir.dt.float32, mybir.dt.int16, mybir.dt.float16
    A = mybir.AluOpType
    nc = bass.Bass()
    u_ext = nc.declare_dram_parameter("u", [tiles, P, F], fp16, isOutput=False)
    o_ext = nc.declare_dram_parameter("out", [tiles, P, F], fp16, isOutput=True)

    # [128,1] constant for the Exp activation bias (const_aps only has 0/1).
    # Synced to ScalarE via a semaphore instead of a barrier so the sync
    # engine can start input DMAs immediately.
    bias_t = nc.alloc_sbuf_tensor("const-ebias", [P, 1], f32)
    e_bias_ap = bias_t.ap()

    ctx = ExitStack()
    ut = [ctx.enter_context(nc.sbuf_tensor(f"ut{j}", [P, F], fp16)) for j in range(NBUF)]
    kq = [ctx.enter_context(nc.sbuf_tensor(f"kq{j}", [P, F], i16)) for j in range(NBUF)]
    vh = [ctx.enter_context(nc.sbuf_tensor(f"vh{j}", [P, F], fp16)) for j in range(NBUF)]
    ys = [ctx.enter_context(nc.sbuf_tensor(f"ys{j}", [P, F], fp16)) for j in range(NBUF)]
    ot = [ctx.enter_context(nc.sbuf_tensor(f"ot{j}", [P, F], fp16)) for j in range(NBUF)]
    # per-buffer-slot DMA semaphores: at most one in-flight DMA per sem, so a
    # waiter on >=16*n can't be satisfied by interleaved partial completions
    # of two DMAs (the 16 per-engine increments of concurrent DMAs interleave).
    s_in = [ctx.enter_context(nc.semaphore(f"s_in{j}")) for j in range(NBUF)]
    s_out = [ctx.enter_context(nc.semaphore(f"s_out{j}")) for j in range(NBUF)]
    s_b = ctx.enter_context(nc.semaphore("s_b"))    # bias memset done
    s_k = ctx.enter_context(nc.semaphore("s_k"))    # DVE K done (per tile)
    s_v = ctx.enter_context(nc.semaphore("s_v"))    # DVE V done (per tile)
    s_y = ctx.enter_context(nc.semaphore("s_y"))    # ScalarE E done (per job)
    s_o = ctx.enter_context(nc.semaphore("s_o"))    # DVE O done (per job)
    block = ctx.enter_context(nc.Block())

    LOOK = NBUF - 1  # input prefetch distance
    last = tiles - 1
    QW = F // NQ  # quarter width for the last tile's drain jobs
    # E/O job counts: tiles 0..last-1 contribute 1 each, the last tile NQ.

    @block.gpsimd
    def _(gpsimd):
        nc.gpsimd.memset(bias_t.ap(), E_BIAS).then_inc(s_b, 1)

    @block.sync
    def _(sync):
        for i in range(min(LOOK, tiles)):
            sync.dma_start(out=ut[i % NBUF][:], in_=u_ext[i]).then_inc(
                s_in[i % NBUF], 16
            )
        for i in range(tiles - LOOK):
            # ut[(i+LOOK)%NBUF] is read by K/V of tile i-1 only; V(i-1) done
            # implies K(i-1) done (same engine, issued earlier).
            if i >= 1:
                sync.wait_ge(s_v, i)
            sync.dma_start(
                out=ut[(i + LOOK) % NBUF][:], in_=u_ext[i + LOOK]
            ).then_inc(s_in[(i + LOOK) % NBUF], 16)

    @block.scalar
    def _(scalar):
        scalar.wait_ge(s_b, 1)  # bias tensor ready (gpsimd memset)
        for i in range(tiles):
            j = i % NBUF
            scalar.wait_ge(s_k, i + 1)  # K(i) done
            if i >= NBUF:
                scalar.wait_ge(s_o, i - NBUF + 1)  # ys slot free (O(i-NBUF))
            if i < last:
                nc.scalar.activation(
                    ys[j][:], kq[j][:], mybir.ActivationFunctionType.Exp,
                    bias=e_bias_ap, scale=E_SCALE,
                ).then_inc(s_y, 1)
                # out-DMA of the previous tile on the ACT HWDGE queue: O(i-1)
                # is all but guaranteed done by the end of E(i), so this
                # rarely stalls and the sync queue free-runs on inputs.
                if i >= 1:
                    scalar.wait_ge(s_o, i)
                    scalar.dma_start(
                        out=o_ext[i - 1], in_=ot[(i - 1) % NBUF][:]
                    ).then_inc(s_out[(i - 1) % NBUF], 16)
            else:
                # drain: E in NQ quarter jobs so O/out-DMA can chase each one
                for q in range(NQ):
                    lo, hi = q * QW, (q + 1) * QW
                    nc.scalar.activation(
                        ys[j][:, lo:hi], kq[j][:, lo:hi],
                        mybir.ActivationFunctionType.Exp,
                        bias=e_bias_ap, scale=E_SCALE,
                    ).then_inc(s_y, 1)
                    if q == 0:
                        scalar.wait_ge(s_o, last)  # O(last-1) done
                        scalar.dma_start(
                            out=o_ext[last - 1], in_=ot[(last - 1) % NBUF][:]
                        ).then_inc(s_out[(last - 1) % NBUF], 16)
                for q in range(NQ):
                    lo, hi = q * QW, (q + 1) * QW
                    scalar.wait_ge(s_o, last + q + 1)  # O quarter q done
                    scalar.dma_start(
                        out=o_ext[last][:, lo:hi], in_=ot[j][:, lo:hi]
                    ).then_inc(s_out[j], 16)

    @block.vector
    def _(vector):
        def stage_kv(i):
            """K(i) + V(i): the tile's DVE quantize + fraction ops."""
            j = i % NBUF
            vector.wait_ge(s_in[j], 16 * (i // NBUF + 1))
            if i >= NBUF:
                vector.wait_ge(s_y, i - NBUF + 1)  # kq slot free (E(i-NBUF))
            nc.vector.tensor_scalar_add(
                out=kq[j][:], in0=ut[j][:], scalar1=K_ADD,
            ).then_inc(s_k, 1)
            # vh slot free: O(i-NBUF) precedes this point on DVE program order
            nc.vector.tensor_tensor(
                out=vh[j][:], in0=ut[j][:], in1=kq[j][:], op=A.subtract,
            ).then_inc(s_v, 1)

        stage_kv(0)
        for i in range(tiles):
            j = i % NBUF
            # issue next tile's K/V before O(i): O waits on E(i), and E(i+1)
            # needs K(i+1) -- this keeps that off the cross-engine cycle.
            if i + 1 < tiles:
                stage_kv(i + 1)
            if i >= NBUF:
                vector.wait_ge(s_out[j], 16 * (i // NBUF))  # ot slot free
            if i < last:
                vector.wait_ge(s_y, i + 1)  # E(i) done
                nc.vector.tensor_tensor(
                    out=ot[j][:], in0=vh[j][:], in1=ys[j][:], op=A.mult,
                ).then_inc(s_o, 1)
            else:
                for q in range(NQ):
                    lo, hi = q * QW, (q + 1) * QW
                    vector.wait_ge(s_y, last + q + 1)  # E quarter q done
                    nc.vector.tensor_tensor(
                        out=ot[j][:, lo:hi], in0=vh[j][:, lo:hi],
                        in1=ys[j][:, lo:hi], op=A.mult,
                    ).then_inc(s_o, 1)

    ctx.close()
    return nc


def _get_nc() -> bass.Bass:
    global _NC
    if _NC is None:
        _NC = _build_nc()
    return _NC


# ------------------------------------------------- exact host-side reference
_XP = np.round(np.linspace(-10.0, 4.0, 17) * 65536.0).astype(np.int64)
_YV = np.round(np.exp(np.linspace(-10.0, 4.0, 17)) * 16384.0).astype(np.int64)
_DY = np.diff(_YV)


def _reference_exact(xs: np.ndarray) -> np.ndarray:
    """Bit-faithful int32 reference for a (small) subset of elements."""
    x_int = np.rint(xs.astype(np.float64) * 65536.0).astype(np.int64)
    mask_low = x_int <= _XP[0]
    mask_high = x_int >= _XP[-1]
    xc = np.clip(x_int, _XP[0], _XP[-1])
    idx = np.clip(np.searchsorted(_XP, xc, side="left") - 1, 0, 15)
    dxv = xc - _XP[idx]
    t_fx = ((dxv << 14) + 28672) // 57344
    prod = t_fx * _DY[idx] + 8192
    pm = prod & 0xFFFFFFFF
    S = np.where(pm >= 1 << 31, pm - (1 << 32), pm)
    interp = _YV[idx] + (S >> 14)
    out_int = np.where(mask_low, _YV[0], np.where(mask_high, _YV[-1], interp))
    return (out_int.astype(np.float32) / np.float32(16384.0)).astype(np.float32)


def _host_fixup(x_flat: np.ndarray, out_flat: np.ndarray) -> None:
    sel = (x_flat >= FIX_HI) | (x_flat < FIX_LO)
    idxs = np.flatnonzero(sel)
    if idxs.size:
        out_flat[idxs] = _reference_exact(x_flat[idxs])


_last_results = None


def kernel(x: np.ndarray) -> np.ndarray:
    assert x.shape == FULL_SHAPE and x.dtype == np.float32, (x.shape, x.dtype)
    nc = _get_nc()
    per = FULL_SHAPE[0] // N_CORES
    u16 = (x * np.float32(8.0 / 7.0) + np.float32(U_BIAS)).astype(np.float16)
    in_maps = [
        {"u": np.ascontiguousarray(u16[i * per : (i + 1) * per]).reshape(TILES, P, F)}
        for i in range(N_CORES)
    ]
    global _last_results
    res = run_bass_kernel_spmd(nc, in_maps, core_ids=list(range(N_CORES)))
    _last_results = res
    out = np.concatenate(
        [
            r["out"].astype(np.float32).reshape(per, FULL_SHAPE[1], FULL_SHAPE[2])
            for r in res.results
        ],
        axis=0,
    )
    _host_fixup(x.ravel(), out.ravel())
    return out


# revision 23
# speedup vs baseline: 1.0441x; 1.0272x over previous
"""Trainium2 Bass kernel for nn_ApproxExp_FXP32in16out14 (histogram_binning).

Reference semantics: fixed-point piecewise-linear LUT approximation of exp(x)
over 17 uniform breakpoints on [-10, 4] (FXP32.16 in, FXP16.14 out), including
int32-wraparound artifacts of the torch reference in segments 14/15.

The reference map factors exactly (up to the per-segment LUT rounding of
~0.35% max) as

    out(x) = rho * exp(0.875*k - 10) * ((z - k) + 0.5 + 1/rho + 1/32768)

with z = (8/7)x + 153/14, k = rne(z), rho = e^0.875 - 1.  The host ships
u = fp16(z + C0), C0 = 0.5 + 1/rho + 1/32768 -- ALL affine constants folded
into one fused scale+cast pass (halving input HBM traffic vs fp32) -- so the
device needs only three DVE ops plus the exp, all in fast 16-bit modes:

  DVE     : K  kq = i16(u - C0)                (TS 4x, rne conversion == k)
            V  v  = u - kq                     (TT 2x)  -> fp16
            O  out = v * ys                    (TT 2x)  -> fp16
  ScalarE : E  ys = Exp(0.875*kq + (ln rho - 10))       -> fp16

v is slaved to kq (v = u - kq), so either rounding direction of the K op at
segment-boundary ties yields a consistent (k, v) pair: the model is exactly
continuous across breakpoints ((1+1/rho)/(e^0.875/rho) == 1), making ties
benign.  Per-core traffic is 64 MiB in + 64 MiB out; the 16 DMA engines
saturate at ~358 GB/s, so the kernel is paced by the DMA window (~375 us)
with DVE (~5.7 us/tile) just under the ~5.9 us/tile DMA cadence.  The last
tile's output is sent as two half-tile DMAs on both HWDGE queues to halve
the pipeline-exposed drain transfer.
Output fp16 (~0.2% rel rms total; the gate is 2e-2), upcast on host.  A
deterministic ~0.3% of elements (int32-wraparound bands at x>=2.7773, the
x>=4 clamp, deep tail x<-4.7) is recomputed exactly on host.

Sharding: pure data parallel, leading dim 64 -> 8 cores x 8.
"""

import math
from contextlib import ExitStack

import numpy as np

import concourse.bass as bass
import concourse.mybir as mybir
from concourse.bass_utils import run_bass_kernel_spmd

# ---------------------------------------------------------------- constants
FULL_SHAPE = (64, 4096, 1024)
N_CORES = 8
TILES, P, F = 64, 128, 4096  # per-core: 64 tiles of [128, 4096]
NBUF = 4

RHO = math.exp(0.875) - 1.0
V_ADD = 0.5 + 1.0 / RHO + 1.0 / 32768.0  # folded into u on host
U_BIAS = 153.0 / 14.0 + V_ADD        # host: u = (8/7)x + U_BIAS
K_ADD = -V_ADD                       # kq = rne(u + K_ADD) == rne(z)
E_SCALE = 0.875
E_BIAS = math.log(RHO) - 10.0        # ys = exp(E_SCALE*k + E_BIAS)

# host-fixup region boundaries (float32 compares on raw x)
FIX_HI = np.float32(2.7773)          # below first int32-wrap threshold (2.77735)
FIX_LO = np.float32(-4.7)            # deep tail: LUT quantization breaks the model

# ------------------------------------------------------------ bass builder
_NC = None


def _build_nc(tiles: int = TILES) -> bass.Bass:
    f32, i16, fp16 = mybir.dt.float32, mybir.dt.int16, mybir.dt.float16
    A = mybir.AluOpType
    nc = bass.Bass()
    u_ext = nc.declare_dram_parameter("u", [tiles, P, F], fp16, isOutput=False)
    o_ext = nc.declare_dram_parameter("out", [tiles, P, F], fp16, isOutput=True)

    # [128,1] constant for the Exp activation bias (const_aps only has 0/1).
    # Synced to ScalarE via a semaphore instead of a barrier so the sync
    # engine can start input DMAs immediately.
    bias_t = nc.alloc_sbuf_tensor("const-ebias", [P, 1], f32)
    e_bias_ap = bias_t.ap()

    ctx = ExitStack()
    ut = [ctx.enter_context(nc.sbuf_tensor(f"ut{j}", [P, F], fp16)) for j in range(NBUF)]
    kq = [ctx.enter_context(nc.sbuf_tensor(f"kq{j}", [P, F], i16)) for j in range(NBUF)]
    vh = [ctx.enter_context(nc.sbuf_tensor(f"vh{j}", [P, F], fp16)) for j in range(NBUF)]
    ys = [ctx.enter_context(nc.sbuf_tensor(f"ys{j}", [P, F], fp16)) for j in range(NBUF)]
    ot = [ctx.enter_context(nc.sbuf_tensor(f"ot{j}", [P, F], fp16)) for j in range(NBUF)]
    # per-buffer-slot DMA semaphores: at most one in-flight DMA per sem, so a
    # waiter on >=16*n can't be satisfied by interleaved partial completions
    # of two DMAs (the 16 per-engine increments of concurrent DMAs interleave).
    s_in = [ctx.enter_context(nc.semaphore(f"s_in{j}")) for j in range(NBUF)]
    s_out = [ctx.enter_context(nc.semaphore(f"s_out{j}")) for j in range(NBUF)]
    s_b = ctx.enter_context(nc.semaphore("s_b"))    # bias memset done
    s_k = ctx.enter_context(nc.semaphore("s_k"))    # DVE K done (per tile)
    s_v = ctx.enter_context(nc.semaphore("s_v"))    # DVE V done (per tile)
    s_y = ctx.enter_context(nc.semaphore("s_y"))    # ScalarE E done (per job)
    s_o = ctx.enter_context(nc.semaphore("s_o"))    # DVE O done (per job)
    block = ctx.enter_context(nc.Block())

    LOOK = NBUF - 1  # input prefetch distance
    last = tiles - 1

    @block.gpsimd
    def _(gpsimd):
        nc.gpsimd.memset(bias_t.ap(), E_BIAS).then_inc(s_b, 1)

    @block.sync
    def _(sync):
        for i in range(min(LOOK, tiles)):
            sync.dma_start(out=ut[i % NBUF][:], in_=u_ext[i]).then_inc(
                s_in[i % NBUF], 16
            )
        for i in range(tiles - LOOK):
            # ut[(i+LOOK)%NBUF] is read by K/V of tile i-1 only; V(i-1) done
            # implies K(i-1) done (same engine, issued earlier).
            if i >= 1:
                sync.wait_ge(s_v, i)
            sync.dma_start(
                out=ut[(i + LOOK) % NBUF][:], in_=u_ext[i + LOOK]
            ).then_inc(s_in[(i + LOOK) % NBUF], 16)
        # drain assist: bottom half of the last tile's output on this queue
        # (the scalar queue sends the top half) to halve the final transfer.
        sync.wait_ge(s_o, tiles)
        sync.dma_start(
            out=o_ext[last][P // 2 :], in_=ot[last % NBUF][P // 2 :, :]
        ).then_inc(s_out[last % NBUF], 16)

    @block.scalar
    def _(scalar):
        scalar.wait_ge(s_b, 1)  # bias tensor ready (gpsimd memset)
        for i in range(tiles):
            j = i % NBUF
            scalar.wait_ge(s_k, i + 1)  # K(i) done
            if i >= NBUF:
                scalar.wait_ge(s_o, i - NBUF + 1)  # ys slot free (O(i-NBUF))
            nc.scalar.activation(
                ys[j][:], kq[j][:], mybir.ActivationFunctionType.Exp,
                bias=e_bias_ap, scale=E_SCALE,
            ).then_inc(s_y, 1)
            # out-DMA of the previous tile on the ACT HWDGE queue: O(i-1)
            # is all but guaranteed done by the end of E(i), so this
            # rarely stalls and the sync queue free-runs on inputs.
            if i >= 1:
                scalar.wait_ge(s_o, i)
                scalar.dma_start(
                    out=o_ext[i - 1], in_=ot[(i - 1) % NBUF][:]
                ).then_inc(s_out[(i - 1) % NBUF], 16)
        # drain: top half of the last tile's output (bottom half on the sync
        # queue) to halve the final, pipeline-exposed transfer.
        scalar.wait_ge(s_o, tiles)
        scalar.dma_start(
            out=o_ext[last][: P // 2], in_=ot[last % NBUF][: P // 2, :]
        ).then_inc(s_out[last % NBUF], 16)

    @block.vector
    def _(vector):
        def stage_kv(i):
            """K(i) + V(i): the tile's DVE quantize + fraction ops."""
            j = i % NBUF
            vector.wait_ge(s_in[j], 16 * (i // NBUF + 1))
            if i >= NBUF:
                vector.wait_ge(s_y, i - NBUF + 1)  # kq slot free (E(i-NBUF))
            nc.vector.tensor_scalar_add(
                out=kq[j][:], in0=ut[j][:], scalar1=K_ADD,
            ).then_inc(s_k, 1)
            # vh slot free: O(i-NBUF) precedes this point on DVE program order
            nc.vector.tensor_tensor(
                out=vh[j][:], in0=ut[j][:], in1=kq[j][:], op=A.subtract,
            ).then_inc(s_v, 1)

        stage_kv(0)
        for i in range(tiles):
            j = i % NBUF
            # issue next tile's K/V before O(i): O waits on E(i), and E(i+1)
            # needs K(i+1) -- this keeps that off the cross-engine cycle.
            if i + 1 < tiles:
                stage_kv(i + 1)
            if i >= NBUF:
                vector.wait_ge(s_out[j], 16 * (i // NBUF))  # ot slot free
            vector.wait_ge(s_y, i + 1)  # E(i) done
            nc.vector.tensor_tensor(
                out=ot[j][:], in0=vh[j][:], in1=ys[j][:], op=A.mult,
            ).then_inc(s_o, 1)

    ctx.close()
    return nc


def _get_nc() -> bass.Bass:
    global _NC
    if _NC is None:
        _NC = _build_nc()
    return _NC


# ------------------------------------------------- exact host-side reference
_XP = np.round(np.linspace(-10.0, 4.0, 17) * 65536.0).astype(np.int64)
_YV = np.round(np.exp(np.linspace(-10.0, 4.0, 17)) * 16384.0).astype(np.int64)
_DY = np.diff(_YV)


def _reference_exact(xs: np.ndarray) -> np.ndarray:
    """Bit-faithful int32 reference for a (small) subset of elements."""
    x_int = np.rint(xs.astype(np.float64) * 65536.0).astype(np.int64)
    mask_low = x_int <= _XP[0]
    mask_high = x_int >= _XP[-1]
    xc = np.clip(x_int, _XP[0], _XP[-1])
    idx = np.clip(np.searchsorted(_XP, xc, side="left") - 1, 0, 15)
    dxv = xc - _XP[idx]
    t_fx = ((dxv << 14) + 28672) // 57344
    prod = t_fx * _DY[idx] + 8192
    pm = prod & 0xFFFFFFFF
    S = np.where(pm >= 1 << 31, pm - (1 << 32), pm)
    interp = _YV[idx] + (S >> 14)
    out_int = np.where(mask_low, _YV[0], np.where(mask_high, _YV[-1], interp))
    return (out_int.astype(np.float32) / np.float32(16384.0)).astype(np.float32)


def _host_fixup(x_flat: np.ndarray, out_flat: np.ndarray) -> None:
    sel = (x_flat >= FIX_HI) | (x_flat < FIX_LO)
    idxs = np.flatnonzero(sel)
    if idxs.size:
        out_flat[idxs] = _reference_exact(x_flat[idxs])


_last_results = None


def kernel(x: np.ndarray) -> np.ndarray:
    assert x.shape == FULL_SHAPE and x.dtype == np.float32, (x.shape, x.dtype)
    nc = _get_nc()
    per = FULL_SHAPE[0] // N_CORES
    u16 = (x * np.float32(8.0 / 7.0) + np.float32(U_BIAS)).astype(np.float16)
    in_maps = [
        {"u": np.ascontiguousarray(u16[i * per : (i + 1) * per]).reshape(TILES, P, F)}
        for i in range(N_CORES)
    ]
    global _last_results
    res = run_bass_kernel_spmd(nc, in_maps, core_ids=list(range(N_CORES)))
    _last_results = res
    out = np.concatenate(
        [
            r["out"].astype(np.float32).reshape(per, FULL_SHAPE[1], FULL_SHAPE[2])
            for r in res.results
        ],
        axis=0,
    )
    _host_fixup(x.ravel(), out.ravel())
    return out


# revision 30
# speedup vs baseline: 1.0689x; 1.0238x over previous
"""Trainium2 Bass kernel for nn_ApproxExp_FXP32in16out14 (histogram_binning).

Reference semantics: fixed-point piecewise-linear LUT approximation of exp(x)
over 17 uniform breakpoints on [-10, 4] (FXP32.16 in, FXP16.14 out), including
int32-wraparound artifacts of the torch reference in segments 14/15.

The reference map factors exactly (up to the per-segment LUT rounding of
~0.35% max) as

    out(x) = rho * exp(0.875*k - 10) * ((z - k) + 0.5 + 1/rho + 1/32768)

with z = (8/7)x + 153/14, k = rne(z), rho = e^0.875 - 1.  The host ships
u = fp16(z + C0), C0 = 0.5 + 1/rho + 1/32768 -- ALL affine constants folded
into one fused scale+cast pass (halving input HBM traffic vs fp32) -- so the
device needs only three DVE ops plus the exp, all in fast 16-bit modes:

  DVE     : K  kq = i16(u - C0)                (TS 4x, rne conversion == k)
            V  v  = u - kq                     (TT 2x)  -> fp16
            O  out = v * ys                    (TT 2x)  -> fp16
  ScalarE : E  ys = Exp(0.875*kq + (ln rho - 10))       -> fp16

v is slaved to kq (v = u - kq), so either rounding direction of the K op at
segment-boundary ties yields a consistent (k, v) pair: the model is exactly
continuous across breakpoints ((1+1/rho)/(e^0.875/rho) == 1), making ties
benign.  Per-core traffic is 64 MiB in + 64 MiB out; the 16 DMA engines
saturate at ~358 GB/s, so the kernel is paced by the DMA window (~375 us)
with DVE (~5.7 us/tile) just under the ~5.9 us/tile DMA cadence.  The last
tile streams in, computes, and drains out as NQ column-quarter jobs spread
over both HWDGE queues, so the input-gated drain tail shrinks from one full
K+E+O+DMA chain (~10 us) to roughly a quarter of it.
Output fp16 (~0.2% rel rms total; the gate is 2e-2), upcast on host.  A
deterministic ~0.3% of elements (int32-wraparound bands at x>=2.7773, the
x>=4 clamp, deep tail x<-4.7) is recomputed exactly on host.

Sharding: pure data parallel, leading dim 64 -> 8 cores x 8.
"""

import math
from contextlib import ExitStack

import numpy as np

import concourse.bass as bass
import concourse.mybir as mybir
from concourse.bass_utils import run_bass_kernel_spmd

# ---------------------------------------------------------------- constants
FULL_SHAPE = (64, 4096, 1024)
N_CORES = 8
TILES, P, F = 64, 128, 4096  # per-core: 64 tiles of [128, 4096]
NBUF = 4
NQ = 4                       # last tile streams/computes/drains in NQ quarters

RHO = math.exp(0.875) - 1.0
V_ADD = 0.5 + 1.0 / RHO + 1.0 / 32768.0  # folded into u on host
U_BIAS = 153.0 / 14.0 + V_ADD        # host: u = (8/7)x + U_BIAS
K_ADD = -V_ADD                       # kq = rne(u + K_ADD) == rne(z)
E_SCALE = 0.875
E_BIAS = math.log(RHO) - 10.0        # ys = exp(E_SCALE*k + E_BIAS)

# host-fixup region boundaries (float32 compares on raw x)
FIX_HI = np.float32(2.7773)          # below first int32-wrap threshold (2.77735)
FIX_LO = np.float32(-4.7)            # deep tail: LUT quantization breaks the model

# ------------------------------------------------------------ bass builder
_NC = None


def _build_nc(tiles: int = TILES) -> bass.Bass:
    f32, i16, fp16 = mybir.dt.float32, mybir.dt.int16, mybir.dt.float16
    A = mybir.AluOpType
    nc = bass.Bass()
    u_ext = nc.declare_dram_parameter("u", [tiles, P, F], fp16, isOutput=False)
    o_ext = nc.declare_dram_parameter("out", [tiles, P, F], fp16, isOutput=True)

    # [128,1] constant for the Exp activation bias (const_aps only has 0/1).
    # Synced to ScalarE via a semaphore instead of a barrier so the sync
    # engine can start input DMAs immediately.
    bias_t = nc.alloc_sbuf_tensor("const-ebias", [P, 1], f32)
    e_bias_ap = bias_t.ap()

    ctx = ExitStack()
    ut = [ctx.enter_context(nc.sbuf_tensor(f"ut{j}", [P, F], fp16)) for j in range(NBUF)]
    kq = [ctx.enter_context(nc.sbuf_tensor(f"kq{j}", [P, F], i16)) for j in range(NBUF)]
    vh = [ctx.enter_context(nc.sbuf_tensor(f"vh{j}", [P, F], fp16)) for j in range(NBUF)]
    ys = [ctx.enter_context(nc.sbuf_tensor(f"ys{j}", [P, F], fp16)) for j in range(NBUF)]
    ot = [ctx.enter_context(nc.sbuf_tensor(f"ot{j}", [P, F], fp16)) for j in range(NBUF)]
    # per-buffer-slot DMA semaphores: at most one in-flight DMA per sem, so a
    # waiter on >=16*n can't be satisfied by interleaved partial completions
    # of two DMAs (the 16 per-engine increments of concurrent DMAs interleave).
    s_in = [ctx.enter_context(nc.semaphore(f"s_in{j}")) for j in range(NBUF)]
    s_out = [ctx.enter_context(nc.semaphore(f"s_out{j}")) for j in range(NBUF)]
    s_b = ctx.enter_context(nc.semaphore("s_b"))    # bias memset done
    s_k = ctx.enter_context(nc.semaphore("s_k"))    # DVE K done (per tile)
    s_v = ctx.enter_context(nc.semaphore("s_v"))    # DVE V done (per tile)
    s_y = ctx.enter_context(nc.semaphore("s_y"))    # ScalarE E done (per job)
    s_o = ctx.enter_context(nc.semaphore("s_o"))    # DVE O done (per job)
    block = ctx.enter_context(nc.Block())

    LOOK = NBUF - 1  # input prefetch distance
    last = tiles - 1
    QW = F // NQ  # quarter width for the last tile's drain jobs

    @block.gpsimd
    def _(gpsimd):
        nc.gpsimd.memset(bias_t.ap(), E_BIAS).then_inc(s_b, 1)

    @block.sync
    def _(sync):
        for i in range(min(LOOK, tiles)):
            sync.dma_start(out=ut[i % NBUF][:], in_=u_ext[i]).then_inc(
                s_in[i % NBUF], 16
            )
        for i in range(tiles - LOOK):
            # ut[(i+LOOK)%NBUF] is read by K/V of tile i-1 only; V(i-1) done
            # implies K(i-1) done (same engine, issued earlier).
            if i >= 1:
                sync.wait_ge(s_v, i)
            if i + LOOK == last:
                # the last tile streams in as NQ column-quarters so its
                # K/E/O pipeline can start before the full tile has landed,
                # shortening the input-gated drain tail
                for q in range(NQ):
                    lo, hi = q * QW, (q + 1) * QW
                    sync.dma_start(
                        out=ut[last % NBUF][:, lo:hi],
                        in_=u_ext[last][:, lo:hi],
                    ).then_inc(s_in[last % NBUF], 16)
            else:
                sync.dma_start(
                    out=ut[(i + LOOK) % NBUF][:], in_=u_ext[i + LOOK]
                ).then_inc(s_in[(i + LOOK) % NBUF], 16)
        # drain assist: odd output quarters of the last tile on this queue
        # (the scalar queue sends the even ones).
        for q in range(1, NQ, 2):
            lo, hi = q * QW, (q + 1) * QW
            sync.wait_ge(s_o, last + q + 1)  # O quarter q done
            sync.dma_start(
                out=o_ext[last][:, lo:hi], in_=ot[last % NBUF][:, lo:hi]
            ).then_inc(s_out[last % NBUF], 16)

    @block.scalar
    def _(scalar):
        scalar.wait_ge(s_b, 1)  # bias tensor ready (gpsimd memset)
        for i in range(last):
            j = i % NBUF
            scalar.wait_ge(s_k, i + 1)  # K(i) done
            if i >= NBUF:
                scalar.wait_ge(s_o, i - NBUF + 1)  # ys slot free (O(i-NBUF))
            nc.scalar.activation(
                ys[j][:], kq[j][:], mybir.ActivationFunctionType.Exp,
                bias=e_bias_ap, scale=E_SCALE,
            ).then_inc(s_y, 1)
            # out-DMA of the previous tile on the ACT HWDGE queue: O(i-1)
            # is all but guaranteed done by the end of E(i), so this
            # rarely stalls and the sync queue free-runs on inputs.
            if i >= 1:
                scalar.wait_ge(s_o, i)
                scalar.dma_start(
                    out=o_ext[i - 1], in_=ot[(i - 1) % NBUF][:]
                ).then_inc(s_out[(i - 1) % NBUF], 16)
        # drain: the last tile runs as NQ column-quarter jobs chasing its
        # quarter input DMAs; even output quarters go on this queue.
        jl = last % NBUF
        scalar.wait_ge(s_o, last - NBUF + 1)  # ys slot free (O(last-NBUF))
        for q in range(NQ):
            lo, hi = q * QW, (q + 1) * QW
            scalar.wait_ge(s_k, last + q + 1)  # K quarter q done
            nc.scalar.activation(
                ys[jl][:, lo:hi], kq[jl][:, lo:hi],
                mybir.ActivationFunctionType.Exp,
                bias=e_bias_ap, scale=E_SCALE,
            ).then_inc(s_y, 1)
            if q == 0:
                scalar.wait_ge(s_o, last)  # O(last-1) done
                scalar.dma_start(
                    out=o_ext[last - 1], in_=ot[(last - 1) % NBUF][:]
                ).then_inc(s_out[(last - 1) % NBUF], 16)
        for q in range(0, NQ, 2):
            lo, hi = q * QW, (q + 1) * QW
            scalar.wait_ge(s_o, last + q + 1)  # O quarter q done
            scalar.dma_start(
                out=o_ext[last][:, lo:hi], in_=ot[jl][:, lo:hi]
            ).then_inc(s_out[jl], 16)

    @block.vector
    def _(vector):
        def stage_kv(i):
            """K(i) + V(i): the tile's DVE quantize + fraction ops."""
            j = i % NBUF
            vector.wait_ge(s_in[j], 16 * (i // NBUF + 1))
            if i >= NBUF:
                vector.wait_ge(s_y, i - NBUF + 1)  # kq slot free (E(i-NBUF))
            nc.vector.tensor_scalar_add(
                out=kq[j][:], in0=ut[j][:], scalar1=K_ADD,
            ).then_inc(s_k, 1)
            # vh slot free: O(i-NBUF) precedes this point on DVE program order
            nc.vector.tensor_tensor(
                out=vh[j][:], in0=ut[j][:], in1=kq[j][:], op=A.subtract,
            ).then_inc(s_v, 1)

        def tail_kv(q):
            """Quarter-width K/V for the last tile, chasing its quarter DMA."""
            jl = last % NBUF
            lo, hi = q * QW, (q + 1) * QW
            vector.wait_ge(s_in[jl], 16 * (last // NBUF) + 16 * (q + 1))
            if q == 0:
                vector.wait_ge(s_y, last - NBUF + 1)  # kq slot free
            nc.vector.tensor_scalar_add(
                out=kq[jl][:, lo:hi], in0=ut[jl][:, lo:hi], scalar1=K_ADD,
            ).then_inc(s_k, 1)
            nc.vector.tensor_tensor(
                out=vh[jl][:, lo:hi], in0=ut[jl][:, lo:hi],
                in1=kq[jl][:, lo:hi], op=A.subtract,
            ).then_inc(s_v, 1)

        def tail_o(q):
            jl = last % NBUF
            lo, hi = q * QW, (q + 1) * QW
            if q == 0:
                vector.wait_ge(s_out[jl], 16 * (last // NBUF))  # ot slot free
            vector.wait_ge(s_y, last + q + 1)  # E quarter q done
            nc.vector.tensor_tensor(
                out=ot[jl][:, lo:hi], in0=vh[jl][:, lo:hi],
                in1=ys[jl][:, lo:hi], op=A.mult,
            ).then_inc(s_o, 1)

        stage_kv(0)
        for i in range(last):
            j = i % NBUF
            # issue next tile's K/V before O(i): O waits on E(i), and E(i+1)
            # needs K(i+1) -- this keeps that off the cross-engine cycle.
            if i + 1 < last:
                stage_kv(i + 1)
            if i >= NBUF:
                vector.wait_ge(s_out[j], 16 * (i // NBUF))  # ot slot free
            vector.wait_ge(s_y, i + 1)  # E(i) done
            nc.vector.tensor_tensor(
                out=ot[j][:], in0=vh[j][:], in1=ys[j][:], op=A.mult,
            ).then_inc(s_o, 1)
        # drain: interleave the last tile's quarter K/V with quarter O's
        tail_kv(0)
        for q in range(1, NQ):
            tail_kv(q)
            tail_o(q - 1)
        tail_o(NQ - 1)

    ctx.close()
    return nc


def _get_nc() -> bass.Bass:
    global _NC
    if _NC is None:
        _NC = _build_nc()
    return _NC


# ------------------------------------------------- exact host-side reference
_XP = np.round(np.linspace(-10.0, 4.0, 17) * 65536.0).astype(np.int64)
_YV = np.round(np.exp(np.linspace(-10.0, 4.0, 17)) * 16384.0).astype(np.int64)
_DY = np.diff(_YV)


def _reference_exact(xs: np.ndarray) -> np.ndarray:
    """Bit-faithful int32 reference for a (small) subset of elements."""
    x_int = np.rint(xs.astype(np.float64) * 65536.0).astype(np.int64)
    mask_low = x_int <= _XP[0]
    mask_high = x_int >= _XP[-1]
    xc = np.clip(x_int, _XP[0], _XP[-1])
    idx = np.clip(np.searchsorted(_XP, xc, side="left") - 1, 0, 15)
    dxv = xc - _XP[idx]
    t_fx = ((dxv << 14) + 28672) // 57344
    prod = t_fx * _DY[idx] + 8192
    pm = prod & 0xFFFFFFFF
    S = np.where(pm >= 1 << 31, pm - (1 << 32), pm)
    interp = _YV[idx] + (S >> 14)
    out_int = np.where(mask_low, _YV[0], np.where(mask_high, _YV[-1], interp))
    return (out_int.astype(np.float32) / np.float32(16384.0)).astype(np.float32)


def _host_fixup(x_flat: np.ndarray, out_flat: np.ndarray) -> None:
    sel = (x_flat >= FIX_HI) | (x_flat < FIX_LO)
    idxs = np.flatnonzero(sel)
    if idxs.size:
        out_flat[idxs] = _reference_exact(x_flat[idxs])


_last_results = None


def kernel(x: np.ndarray) -> np.ndarray:
    assert x.shape == FULL_SHAPE and x.dtype == np.float32, (x.shape, x.dtype)
    nc = _get_nc()
    per = FULL_SHAPE[0] // N_CORES
    u16 = (x * np.float32(8.0 / 7.0) + np.float32(U_BIAS)).astype(np.float16)
    in_maps = [
        {"u": np.ascontiguousarray(u16[i * per : (i + 1) * per]).reshape(TILES, P, F)}
        for i in range(N_CORES)
    ]
    global _last_results
    res = run_bass_kernel_spmd(nc, in_maps, core_ids=list(range(N_CORES)))
    _last_results = res
    out = np.concatenate(
        [
            r["out"].astype(np.float32).reshape(per, FULL_SHAPE[1], FULL_SHAPE[2])
            for r in res.results
        ],
        axis=0,
    )
    _host_fixup(x.ravel(), out.ravel())
    return out
